# revision 23
# baseline (speedup 1.0000x reference)
"""GraphSAGE 3-layer + output projection on 8 Trainium2 NeuronCores.

Sharding: nodes (and dst-partitioned edges) split across 8 cores, 1280
nodes/core (N padded 10000->10240). Per layer: all cores hold the full
previous-layer activations in DRAM; each core gathers its edges' unique
source rows with batched dma_gather ops (SWDGE Q7 descriptor generation
is the wall at ~8ns/row, so rows are deduped per dst tile and pad slots
are skipped via trailing -1 indices), segment-sums them on the
TensorEngine via host-built count-matrix matmuls (bf16, free dim 512),
scales by 1/deg, transposes to feature-major, and applies lin_l/lin_r
as fp32r matmuls. Activations are AllGathered in 3 pieces; each tile's
gather is split into an op1 (sources in AG pieces A+B) that can start
as soon as piece B of the previous layer lands, and a small op2
(sources anywhere) that waits for piece C — overlapping the gather
stream across layer boundaries (pa bufs=4 keeps 4 tiles' PSUM
accumulations open across the boundary).
"""
import sys, types, ctypes, contextlib

import numpy as np


def _install_ntff_hook():
    # antenv.axon_hooks is missing in this image; provide it so
    # bass_utils trace=True can profile via libaxon_pjrt.so.
    if "antenv.axon_hooks" in sys.modules:
        return
    try:
        import antenv  # noqa: F401
    except ImportError:
        return
    mod = types.ModuleType("antenv.axon_hooks")
    state = {"hook": None}
    mod.set_axon_ntff_profile_hook = lambda h: state.__setitem__("hook", h)
    mod.get_axon_ntff_profile_hook = lambda: state["hook"]
    sys.modules["antenv.axon_hooks"] = mod
    try:
        lib = ctypes.CDLL('/opt/axon/libaxon_pjrt.so')
    except OSError:
        return
    if not hasattr(lib, "axon_start_nrt_profile"):
        return
    lib.axon_start_nrt_profile.argtypes = [ctypes.POINTER(ctypes.c_int64), ctypes.c_size_t]
    lib.axon_start_nrt_profile.restype = ctypes.c_int64
    lib.axon_stop_nrt_profile.argtypes = [ctypes.c_char_p]
    lib.axon_stop_nrt_profile.restype = ctypes.c_int64

    @contextlib.contextmanager
    def _hook(output_dir, device_ids):
        import jax
        jax.devices()
        if device_ids:
            ids = (ctypes.c_int64 * len(device_ids))(*device_ids)
            rc = lib.axon_start_nrt_profile(ids, len(device_ids))
        else:
            rc = lib.axon_start_nrt_profile(None, 0)
        if rc != 0:
            raise RuntimeError(f"axon_start_nrt_profile rc={rc}")
        try:
            yield
        finally:
            n = lib.axon_stop_nrt_profile(str(output_dir).encode())
            print(f"profile: {n} file(s) written to {output_dir}", file=sys.stderr)

    state["hook"] = _hook


_install_ntff_hook()

import concourse.bass2jax as _b2j
_orig_cc_hook = _b2j.neuronx_cc_hook
def _dbg_cc_hook(*a, **kw):
    try:
        return _orig_cc_hook(*a, **kw)
    except BaseException:
        import traceback
        traceback.print_exc()
        raise
_b2j.neuronx_cc_hook = _dbg_cc_hook

import concourse.hw_specs as _hw_specs
# calibrate the tile scheduler's SWDGE model to the measured dma_gather
# descriptor-generation rate (~8 ns/idx) so simulated schedules match HW
_hw_specs.TRN2Spec.SWDGE_NS_PER_DESCRIPTOR = 8.0

import concourse.bass as bass
import concourse.tile as tile
from concourse import mybir, bacc
from concourse.bass_utils import run_bass_kernel_spmd

F32 = mybir.dt.float32
F32R = mybir.dt.float32r
BF16 = mybir.dt.bfloat16
I32 = mybir.dt.int32
I16 = mybir.dt.int16

N, D, H, O = 10000, 512, 512, 128
C = 8              # cores
NP = 10240         # padded node count
NCORE = NP // C    # 1280 nodes per core
NT = NCORE // 128  # 10 dst tiles per core
GROUPS = [(0, 4), (4, 8), (8, 10)]  # dense/AG groups by dst tile range
AB_ROWS = 8192     # xg rows covered by AllGather pieces A+B


NGT = 8            # tiles 0..NGT-1 aggregate via gather; NGT..NT-1 via
                   # dense count-matrix matmuls on the TensorEngine
NBLK = NP // 128   # 80 source blocks for the dense-adjacency path


def _host_prep(x, edge_index):
    src = np.asarray(edge_index[0], dtype=np.int64)
    dst = np.asarray(edge_index[1], dtype=np.int64)
    deg = np.bincount(dst, minlength=NP).astype(np.float64)
    invdeg = (1.0 / np.maximum(deg, 1.0)).astype(np.float32)

    # piece-wise AllGather layout for layers 1,2: node (c, loc) lives at
    # row c*512+loc (loc<512), 4096+c*512+(loc-512), or 8192+c*256.
    # x0 is stored in the same remapped order so the same gather indices,
    # and the same per-block count table, serve all three layers.
    allnodes = np.arange(NP, dtype=np.int64)
    cc, loc = allnodes // NCORE, allnodes % NCORE
    remap = np.where(
        loc < 512, cc * 512 + loc,
        np.where(loc < 1024, 4096 + cc * 512 + (loc - 512),
                 8192 + cc * 256 + (loc - 1024))).astype(np.int64)

    # per (core, gathered dst tile): unique sources split into op1
    # (remap < AB_ROWS) and op2 (remap >= AB_ROWS); oh[slot, dstoff] = count
    uniq = [[None] * NGT for _ in range(C)]
    n1 = np.zeros((C, NGT), np.int64)
    n2 = np.zeros((C, NGT), np.int64)
    for c in range(C):
        for t in range(NGT):
            g = c * NT + t
            sel = (dst >= g * 128) & (dst < (g + 1) * 128)
            s_e = src[sel]
            d_e = (dst[sel] - g * 128).astype(np.int64)
            us = np.unique(s_e)
            m1 = remap[us] < AB_ROWS
            us1, us2 = us[m1], us[~m1]
            uniq[c][t] = (us1, us2, s_e, d_e)
            n1[c, t], n2[c, t] = len(us1), len(us2)

    cnt1m = n1.max(axis=0)
    cnt2m = n2.max(axis=0)
    T1 = np.maximum(np.ceil(cnt1m / 128).astype(np.int64), 1)
    T2 = np.maximum(np.ceil(cnt2m / 128).astype(np.int64), 1)
    T = T1 + T2
    bases = np.concatenate([[0], np.cumsum(T)])[:-1]
    ST = int(T.sum())

    gidx12 = np.full((C, 128, ST * 8), -1, np.int16)  # xg/x0r rows
    ohv = np.zeros((C, 128, ST, 128), np.float32)     # [slot, sub, dstoff]

    def fill(tbl, vals, col0):
        # value i lands at gather position col0*128 + i; idx table wraps
        # position j at [j % 16, j // 16], replicated over 8 row groups.
        if len(vals) == 0:
            return
        i = np.arange(len(vals))
        pos = col0 * 128 + i
        for r in range(8):
            tbl[16 * r + pos % 16, pos // 16] = vals

    for c in range(C):
        for t in range(NGT):
            us1, us2, s_e, d_e = uniq[c][t]
            b = int(bases[t])
            k1 = int(T1[t])
            # pad each op's index list to the cross-core max with idx 0
            # (the -1 tail after that is skipped via num_idxs_reg)
            p1 = np.zeros(int(cnt1m[t]), np.int64)
            p1[:len(us1)] = us1
            p2 = np.zeros(int(cnt2m[t]), np.int64)
            p2[:len(us2)] = us2
            fill(gidx12[c], remap[p1].astype(np.int16), b)
            fill(gidx12[c], remap[p2].astype(np.int16), b + k1)
            # slot of each unique src: op1 slots then op2 slots (op2 slots
            # continue at subtile offset k1, position restarts at 0 there)
            slot_of = {}
            for i, s in enumerate(us1):
                slot_of[int(s)] = (b + i // 128, i % 128)
            for i, s in enumerate(us2):
                slot_of[int(s)] = (b + k1 + i // 128, i % 128)
            for s_i, d_i in zip(s_e, d_e):
                sub, p = slot_of[int(s_i)]
                ohv[c, p, sub, d_i] += 1.0

    # dense-adjacency count table for tiles NGT..NT-1:
    # cntv[c, p, blk, (t-NGT)*128+dstoff] = #edges(node remap'd to
    # blk*128+p -> dst (c*NT+t)*128+dstoff)
    NAT = NT - NGT
    cntv = np.zeros((C, 128, NBLK, NAT * 128), np.float32)
    rsrc = remap[src]
    for c in range(C):
        for t in range(NGT, NT):
            g = c * NT + t
            sel = (dst >= g * 128) & (dst < (g + 1) * 128)
            s_e = rsrc[sel]
            d_e = (dst[sel] - g * 128).astype(np.int64)
            np.add.at(cntv[c], (s_e % 128, s_e // 128,
                                (t - NGT) * 128 + d_e), 1.0)

    x_pad = np.zeros((NP, D), np.float32)
    x_pad[:N] = np.asarray(x, dtype=np.float32)
    x0r = np.zeros((NP, D), np.float32)
    x0r[remap] = x_pad

    invdeg_sb = np.empty((C, 128, NT), np.float32)
    for c in range(C):
        invdeg_sb[c] = invdeg[c * NCORE:(c + 1) * NCORE].reshape(NT, 128).T

    xT0 = np.empty((C, 128, 4, NCORE), np.float32)
    for c in range(C):
        xT0[c] = x_pad[c * NCORE:(c + 1) * NCORE].reshape(NCORE, 4, 128).transpose(2, 1, 0)

    import ml_dtypes
    ohv = ohv.astype(ml_dtypes.bfloat16)
    cntv = cntv.astype(ml_dtypes.bfloat16)
    return (x0r, gidx12, ohv, cntv, invdeg_sb, xT0,
            T1, T2, bases, ST, cnt1m, cnt2m)


def _wsb(w):
    # [K, M] -> SBUF layout [128, K/128, M], bf16
    import ml_dtypes
    w = np.asarray(w, np.float32)
    return np.ascontiguousarray(
        w.reshape(w.shape[0] // 128, 128, w.shape[1]).transpose(1, 0, 2)
    ).astype(ml_dtypes.bfloat16)


def _bsb(b):
    # [M] -> [128, M/128]
    b = np.asarray(b, np.float32)
    return np.ascontiguousarray(b.reshape(b.shape[0] // 128, 128).T)


def _build_program(T1, T2, bases, ST, cnt1m, cnt2m):
    T1MAX, T2MAX = int(T1.max()), int(T2.max())
    nc = bacc.Bacc(None, target_bir_lowering=False, debug=False, num_devices=C,
                   dynamic_dma_scratch_size=16384)

    x0_d = nc.declare_dram_parameter("x_full0", [NP, D], BF16, isOutput=False)
    gidx12_d = nc.declare_dram_parameter("gidx12", [128, ST * 8], I16, isOutput=False)
    oh_d = nc.declare_dram_parameter("ohv", [128, ST, 128], BF16, isOutput=False)
    cnt_d = nc.declare_dram_parameter("cntv", [128, NBLK, (NT - NGT) * 128], BF16,
                                      isOutput=False)
    invdeg_d = nc.declare_dram_parameter("invdeg", [128, NT], F32, isOutput=False)
    ident_d = nc.declare_dram_parameter("ident", [128, 128], F32, isOutput=False)
    identb_d = nc.declare_dram_parameter("identb", [128, 128], BF16, isOutput=False)
    zeros_d = nc.declare_dram_parameter("zeros", [128, 1], F32, isOutput=False)
    xT0_d = nc.declare_dram_parameter("xT0", [128, 4, NCORE], BF16, isOutput=False)
    w_d = {}
    for l in range(3):
        w_d[f"wl{l}"] = nc.declare_dram_parameter(f"wl{l}", [128, 4, H], BF16, isOutput=False)
        w_d[f"wr{l}"] = nc.declare_dram_parameter(f"wr{l}", [128, 4, H], BF16, isOutput=False)
        w_d[f"b{l}"] = nc.declare_dram_parameter(f"b{l}", [128, 4], F32, isOutput=False)
    wout_d = nc.declare_dram_parameter("wout", [128, 4, O], BF16, isOutput=False)
    bout_d = nc.declare_dram_parameter("bout", [128, 1], F32, isOutput=False)
    out_d = nc.declare_dram_parameter("out", [NCORE, O], F32, isOutput=True)

    xg = [None, nc.dram_tensor("xg1", [NP, D], BF16, addr_space="Shared"),
          nc.dram_tensor("xg2", [NP, D], BF16, addr_space="Shared")]
    xc = [None, nc.dram_tensor("xc1", [NCORE, D], BF16),
          nc.dram_tensor("xc2", [NCORE, D], BF16)]

    with tile.TileContext(nc) as tc:
        with tc.tile_pool(name="const", bufs=1) as constp, \
             tc.tile_pool(name="xT", bufs=2) as xTp, \
             tc.tile_pool(name="aggT", bufs=1) as aggTp, \
             tc.tile_pool(name="xs1", bufs=2) as xs1p, \
             tc.tile_pool(name="xs2", bufs=2) as xs2p, \
             tc.tile_pool(name="oh1", bufs=2) as oh1p, \
             tc.tile_pool(name="oh2", bufs=2) as oh2p, \
             tc.tile_pool(name="agg", bufs=2) as aggp, \
             tc.tile_pool(name="xnm", bufs=2) as xnmp, \
             tc.tile_pool(name="wts", bufs=2) as wp, \
             tc.tile_pool(name="xall", bufs=3) as xallp, \
             tc.tile_pool(name="dmy", bufs=3) as dmyp, \
             tc.tile_pool(name="pa", bufs=3, space="PSUM") as pap, \
             tc.tile_pool(name="padj", bufs=1, space="PSUM") as padjp, \
             tc.tile_pool(name="pt", bufs=1, space="PSUM") as ptp, \
             tc.tile_pool(name="pd", bufs=1, space="PSUM") as pdp:

            # ---- load constants ----
            gidx12_sb = constp.tile([128, ST * 8], I16)
            nc.sync.dma_start(gidx12_sb[:], gidx12_d[:])
            cnt_sb = constp.tile([128, NBLK, (NT - NGT) * 128], BF16)
            nc.sync.dma_start(cnt_sb[:], cnt_d[:])
            invdeg_sb = constp.tile([128, NT], F32)
            nc.sync.dma_start(invdeg_sb[:], invdeg_d[:])
            ident = constp.tile([128, 128], F32)
            nc.sync.dma_start(ident[:], ident_d[:])
            identb = constp.tile([128, 128], BF16)
            nc.sync.dma_start(identb[:], identb_d[:])
            wsb = {}
            for l in range(3):
                wsb[f"b{l}"] = constp.tile([128, 4], F32, name=f"bsb{l}")
                nc.sync.dma_start(wsb[f"b{l}"][:], w_d[f"b{l}"][:])
            wout_sb = constp.tile([128, 4, O], BF16)
            nc.sync.dma_start(wout_sb[:], wout_d[:])
            bout_sb = constp.tile([128, 1], F32)
            nc.sync.dma_start(bout_sb[:], bout_d[:])

            xT_cur = xTp.tile([128, 4, NCORE], BF16)
            nc.sync.dma_start(xT_cur[:], xT0_d[:])

            gate_in = None
            for l in range(3):
                gidx = gidx12_sb
                aggT = aggTp.tile([128, 4, NCORE], BF16)
                xT_next = xTp.tile([128, 4, NCORE], BF16)
                wlr = wp.tile([128, 8, H], BF16, name="wlr")
                nc.sync.dma_start(wlr[:, 0:4, :], w_d[f"wl{l}"][:])
                nc.sync.dma_start(wlr[:, 4:8, :], w_d[f"wr{l}"][:])
                wl, wr, bb = wlr[:, 0:4, :], wlr[:, 4:8, :], wsb[f"b{l}"]

                pa_of = {}

                # tiles NGT..NT-1: dense-adjacency aggregation on the
                # TensorEngine — psum[dst, :] += cnt_blk[src, dst]^T @ x_blk.
                # Runs off streamed source blocks (no gather); doubles as
                # PE keep-busy filler between gather-gated count matmuls.
                padj = [padjp.tile([128, D], F32, name=f"padj{t}")
                        for t in range(NGT, NT)]

                def do_adj(b0, b1):
                    # source blocks in batches of 4 per DMA (one [512, D]
                    # slab loaded as [128, 4, D]), issued on the Scalar
                    # queue to keep the Sync queue free for oh/xc traffic
                    src_d = x0_d if l == 0 else xg[l]
                    for b4 in range(b0 // 4, b1 // 4):
                        xall = xallp.tile([128, 4, D], BF16, name="xall")
                        nc.scalar.dma_start(
                            xall[:],
                            src_d[b4 * 512:(b4 + 1) * 512, :].rearrange(
                                "(j p) f -> p j f", p=128))
                        for j in range(4):
                            b = b4 * 4 + j
                            for t in range(NT - NGT):
                                nc.tensor.matmul(
                                    padj[t][:],
                                    lhsT=cnt_sb[:, b, t * 128:(t + 1) * 128],
                                    rhs=xall[:, j, :],
                                    start=(b == 0), stop=(b == NBLK - 1))

                def do_adj_tail(t):
                    agg = aggp.tile([128, D], F32, name="agg")
                    nc.scalar.activation(
                        agg[:], padj[t - NGT][:],
                        mybir.ActivationFunctionType.Copy,
                        scale=invdeg_sb[:, t:t + 1])
                    for k in range(4):
                        pt = ptp.tile([128, 128], F32, name="pt")
                        nc.tensor.transpose(pt[:], agg[:, k * 128:(k + 1) * 128], ident[:])
                        nc.vector.tensor_copy(aggT[:, k, t * 128:(t + 1) * 128], pt[:])

                def do_op1(t, gate=None):
                    # gather op1 (unique srcs in AG pieces A+B) + matmuls;
                    # the PSUM accumulation group stays open for op2. The
                    # zero-fill reads `gate` (written right after an AG
                    # trigger on the Pool queue) to pin scheduling order.
                    ne = int(T1[t])
                    nreal = int(cnt1m[t])
                    b = int(bases[t])
                    xs = xs1p.tile([128, T1MAX, D], BF16, name="xs1")
                    nc.vector.memset(xs[:, ne - 1, :], 0.0)
                    src_ap = x0_d[:] if l == 0 else xg[l][0:AB_ROWS, :]
                    nc.gpsimd.dma_gather(
                        out_ap=xs[:, :ne, :], in_ap=src_ap,
                        idxs_ap=gidx[:, b * 8:(b + ne) * 8],
                        num_idxs=ne * 128, num_idxs_reg=nreal,
                        elem_size=D, single_packet=False)
                    oh = oh1p.tile([128, T1MAX, 128], BF16, name="oh1")
                    nc.sync.dma_start(oh[:, :ne, :], oh_d[:, b:b + ne, :])
                    pa = pap.tile([128, D], F32, name="pa")
                    pa_of[t] = pa
                    for e in range(ne):
                        nc.tensor.matmul(
                            pa[:], lhsT=oh[:, e, :], rhs=xs[:, e, :],
                            start=(e == 0), stop=False)

                def do_op2(t, gate=None):
                    # gather op2 (srcs needing AG piece C), close the
                    # accumulation, scale by 1/deg, transpose to feat-major.
                    ne = int(T2[t])
                    nreal = int(cnt2m[t])
                    b = int(bases[t]) + int(T1[t])
                    xs = xs2p.tile([128, T2MAX, D], BF16, name="xs2")
                    nc.vector.memset(xs[:, ne - 1, :], 0.0)
                    src_ap = x0_d[:] if l == 0 else xg[l][:]
                    nc.gpsimd.dma_gather(
                        out_ap=xs[:, :ne, :], in_ap=src_ap,
                        idxs_ap=gidx[:, b * 8:(b + ne) * 8],
                        num_idxs=ne * 128, num_idxs_reg=nreal,
                        elem_size=D, single_packet=False)
                    oh = oh2p.tile([128, T2MAX, 128], BF16, name="oh2")
                    nc.sync.dma_start(oh[:, :ne, :], oh_d[:, b:b + ne, :])
                    pa = pa_of.pop(t)
                    for e in range(ne):
                        nc.tensor.matmul(
                            pa[:], lhsT=oh[:, e, :], rhs=xs[:, e, :],
                            start=False, stop=(e == ne - 1))
                    agg = aggp.tile([128, D], F32, name="agg")
                    nc.scalar.activation(
                        agg[:], pa[:], mybir.ActivationFunctionType.Copy,
                        scale=invdeg_sb[:, t:t + 1])
                    for k in range(4):
                        pt = ptp.tile([128, 128], F32, name="pt")
                        nc.tensor.transpose(pt[:], agg[:, k * 128:(k + 1) * 128], ident[:])
                        nc.vector.tensor_copy(aggT[:, k, t * 128:(t + 1) * 128], pt[:])

                def do_dense_group(goff, gsz):
                    for m in range(4):
                        pd = pdp.tile([128, 512], F32, name="pd")
                        for k in range(4):
                            nc.tensor.matmul(
                                pd[:, :gsz],
                                lhsT=wl[:, k, m * 128:(m + 1) * 128],
                                rhs=aggT[:, k, goff:goff + gsz],
                                start=(k == 0), stop=False)
                        for k in range(4):
                            nc.tensor.matmul(
                                pd[:, :gsz],
                                lhsT=wr[:, k, m * 128:(m + 1) * 128],
                                rhs=xT_cur[:, k, goff:goff + gsz],
                                start=False, stop=(k == 3))
                        nc.scalar.activation(
                            xT_next[:, m, goff:goff + gsz], pd[:, :gsz],
                            mybir.ActivationFunctionType.Relu,
                            bias=bb[:, m:m + 1])
                    if l < 2:
                        for t in range(goff // 128, (goff + gsz) // 128):
                            xnm = xnmp.tile([128, D], BF16, name="xnm")
                            for k in range(4):
                                pt = ptp.tile([128, 128], BF16, name="ptx")
                                nc.tensor.transpose(
                                    pt[:], xT_next[:, k, t * 128:(t + 1) * 128],
                                    identb[:])
                                nc.vector.tensor_copy(xnm[:, k * 128:(k + 1) * 128], pt[:])
                            nc.sync.dma_start(xc[l + 1][t * 128:(t + 1) * 128, :], xnm[:])

                def do_final_group(t0, t1, xT_src):
                    goff, gsz = t0 * 128, (t1 - t0) * 128
                    pd = pdp.tile([128, 512], F32, name="pd")
                    for k in range(4):
                        nc.tensor.matmul(
                            pd[:, :gsz],
                            lhsT=wout_sb[:, k, :],
                            rhs=xT_src[:, k, goff:goff + gsz],
                            start=(k == 0), stop=(k == 3))
                    oT = aggp.tile([128, 512], F32)
                    nc.scalar.activation(
                        oT[:, :gsz], pd[:, :gsz],
                        mybir.ActivationFunctionType.Identity, bias=bout_sb[:, 0:1])
                    for tt in range(gsz // 128):
                        t = t0 + tt
                        pt = ptp.tile([128, 128], F32)
                        nc.tensor.transpose(pt[:], oT[:, tt * 128:(tt + 1) * 128], ident[:])
                        onm = xnmp.tile([128, O], F32)
                        nc.vector.tensor_copy(onm[:], pt[:])
                        nc.sync.dma_start(out_d[t * 128:(t + 1) * 128, :], onm[:])

                # Pool-queue order: each AG trigger is emitted only after the
                # NEXT group's op1 gathers, so a trigger blocked on its dense
                # group can never starve the gather stream (no head-of-line
                # cycle), while still firing ~2 groups earlier than end-of-layer.
                ag_bounds = [(0, 4096), (4096, 8192), (8192, 10240)]

                def do_ag(gi):
                    t0, t1 = GROUPS[gi]
                    with tc.high_priority():
                        do_dense_group(t0 * 128, (t1 - t0) * 128)
                        if l < 2:
                            lo, hi = ag_bounds[gi]
                            nc.gpsimd.collective_compute(
                                "AllGather", mybir.AluOpType.bypass,
                                replica_groups=[list(range(C))],
                                ins=[xc[l + 1][t0 * 128:t1 * 128, :]],
                                outs=[xg[l + 1][lo:hi, :]])
                            return None
                        do_final_group(t0, t1, xT_next)
                    return None

                # Pool-queue order: each AG trigger is emitted right after the
                # minimal gather set its dense group depends on (plus one op1
                # of slack to absorb the dense-chain latency). Adjacency
                # matmul blocks are interleaved between gather-gated work so
                # the TensorEngine stays busy (HAM stays un-throttled); blocks
                # 64-79 need AG piece C so they are emitted after do_ag(1).
                for t in range(0, 4):
                    do_op1(t)
                    do_adj(t * 8, (t + 1) * 8)
                do_op1(4)
                for t in range(0, 4):
                    do_op2(t)
                do_ag(0)
                for i, t in enumerate(range(5, 8)):
                    do_op1(t)
                    do_adj(32 + 12 * i, min(32 + 12 * (i + 1), 64))
                for t in range(4, 8):
                    do_op2(t)
                do_ag(1)
                do_adj(64, NBLK)
                for t in range(NGT, NT):
                    do_adj_tail(t)
                do_ag(2)
                xT_cur = xT_next

    nc.compile()
    return nc


def _run(inputs, trace=False):
    x = inputs["x"]
    edge_index = inputs["edge_index"]
    (x0r, gidx12, ohv, cntv, invdeg_sb, xT0,
     T1, T2, bases, ST, cnt1m, cnt2m) = _host_prep(x, edge_index)
    nc = _build_program(T1, T2, bases, ST, cnt1m, cnt2m)

    import ml_dtypes
    shared = {
        "x_full0": x0r.astype(ml_dtypes.bfloat16),
        "wout": _wsb(inputs["w_out"]),
        "bout": np.asarray(inputs["b_out"], np.float32).reshape(128, 1),
        "ident": np.eye(128, dtype=np.float32),
        "identb": np.eye(128, dtype=np.float32).astype(ml_dtypes.bfloat16),
        "zeros": np.zeros((128, 1), np.float32),
    }
    for l in range(3):
        shared[f"wl{l}"] = _wsb(inputs[f"w_l{l}"])
        shared[f"wr{l}"] = _wsb(inputs[f"w_r{l}"])
        shared[f"b{l}"] = _bsb(inputs[f"b_l{l}"])

    in_maps = []
    for c in range(C):
        m = dict(shared)
        m["gidx12"] = np.ascontiguousarray(gidx12[c])
        m["cntv"] = np.ascontiguousarray(cntv[c])
        m["ohv"] = np.ascontiguousarray(ohv[c])
        m["invdeg"] = np.ascontiguousarray(invdeg_sb[c])
        m["xT0"] = np.ascontiguousarray(xT0[c]).astype(ml_dtypes.bfloat16)
        in_maps.append(m)

    res = run_bass_kernel_spmd(nc, in_maps, list(range(C)), trace=trace)
    out = np.concatenate([res.results[c]["out"] for c in range(C)], axis=0)[:N]
    return out.astype(np.float32), res


def kernel(**inputs):
    out, _ = _run(inputs, trace=False)
    return out


def kernel_timed(**inputs):
    out, res = _run(inputs, trace=True)
    return out, res



# revision 32
# speedup vs baseline: 1.1070x; 1.1070x over previous
"""GraphSAGE 3-layer + output projection on 8 Trainium2 NeuronCores.

Sharding: nodes (and dst-partitioned edges) split across 8 cores, 1280
nodes/core (N padded 10000->10240). Per layer: all cores hold the full
previous-layer activations in DRAM; each core gathers its edges' unique
source rows with batched dma_gather ops (SWDGE Q7 descriptor generation
is the wall at ~8ns/row, so rows are deduped per dst tile and pad slots
are skipped via trailing -1 indices), segment-sums them on the
TensorEngine via host-built count-matrix matmuls (bf16, free dim 512),
scales by 1/deg, transposes to feature-major, and applies lin_l/lin_r
as fp32r matmuls. Activations are AllGathered in 3 pieces; each tile's
gather is split into an op1 (sources in AG pieces A+B) that can start
as soon as piece B of the previous layer lands, and a small op2
(sources anywhere) that waits for piece C — overlapping the gather
stream across layer boundaries (pa bufs=4 keeps 4 tiles' PSUM
accumulations open across the boundary).
"""
import sys, types, ctypes, contextlib

import numpy as np


def _install_ntff_hook():
    # antenv.axon_hooks is missing in this image; provide it so
    # bass_utils trace=True can profile via libaxon_pjrt.so.
    if "antenv.axon_hooks" in sys.modules:
        return
    try:
        import antenv  # noqa: F401
    except ImportError:
        return
    mod = types.ModuleType("antenv.axon_hooks")
    state = {"hook": None}
    mod.set_axon_ntff_profile_hook = lambda h: state.__setitem__("hook", h)
    mod.get_axon_ntff_profile_hook = lambda: state["hook"]
    sys.modules["antenv.axon_hooks"] = mod
    try:
        lib = ctypes.CDLL('/opt/axon/libaxon_pjrt.so')
    except OSError:
        return
    if not hasattr(lib, "axon_start_nrt_profile"):
        return
    lib.axon_start_nrt_profile.argtypes = [ctypes.POINTER(ctypes.c_int64), ctypes.c_size_t]
    lib.axon_start_nrt_profile.restype = ctypes.c_int64
    lib.axon_stop_nrt_profile.argtypes = [ctypes.c_char_p]
    lib.axon_stop_nrt_profile.restype = ctypes.c_int64

    @contextlib.contextmanager
    def _hook(output_dir, device_ids):
        import jax
        jax.devices()
        if device_ids:
            ids = (ctypes.c_int64 * len(device_ids))(*device_ids)
            rc = lib.axon_start_nrt_profile(ids, len(device_ids))
        else:
            rc = lib.axon_start_nrt_profile(None, 0)
        if rc != 0:
            raise RuntimeError(f"axon_start_nrt_profile rc={rc}")
        try:
            yield
        finally:
            n = lib.axon_stop_nrt_profile(str(output_dir).encode())
            print(f"profile: {n} file(s) written to {output_dir}", file=sys.stderr)

    state["hook"] = _hook


_install_ntff_hook()

import concourse.bass2jax as _b2j
_orig_cc_hook = _b2j.neuronx_cc_hook
def _dbg_cc_hook(*a, **kw):
    try:
        return _orig_cc_hook(*a, **kw)
    except BaseException:
        import traceback
        traceback.print_exc()
        raise
_b2j.neuronx_cc_hook = _dbg_cc_hook

import concourse.hw_specs as _hw_specs
# calibrate the tile scheduler's SWDGE model to the measured dma_gather
# descriptor-generation rate (~8 ns/idx) so simulated schedules match HW
_hw_specs.TRN2Spec.SWDGE_NS_PER_DESCRIPTOR = 8.0

import concourse.bass as bass
import concourse.tile as tile
from concourse import mybir, bacc
from concourse.bass_utils import run_bass_kernel_spmd

F32 = mybir.dt.float32
F32R = mybir.dt.float32r
BF16 = mybir.dt.bfloat16
I32 = mybir.dt.int32
I16 = mybir.dt.int16

N, D, H, O = 10000, 512, 512, 128
C = 8              # cores
NP = 10240         # padded node count
NCORE = NP // C    # 1280 nodes per core
NT = NCORE // 128  # 10 dst tiles per core
GROUPS = [(0, 4), (4, 8), (8, 10)]  # dense/AG groups by dst tile range
AB_ROWS = 8192     # xg rows covered by AllGather pieces A+B


A_ROWS = 4096      # xg rows covered by AllGather piece A


def _host_prep(x, edge_index):
    src = np.asarray(edge_index[0], dtype=np.int64)
    dst = np.asarray(edge_index[1], dtype=np.int64)
    deg = np.bincount(dst, minlength=NP).astype(np.float64)
    invdeg = (1.0 / np.maximum(deg, 1.0)).astype(np.float32)

    # piece-wise AllGather layout for layers 1,2: node (c, loc) lives at
    # row c*512+loc (loc<512), 4096+c*512+(loc-512), or 8192+c*256.
    # x0 is stored in the same remapped order so the same gather indices
    # serve all three layers.
    allnodes = np.arange(NP, dtype=np.int64)
    cc, loc = allnodes // NCORE, allnodes % NCORE
    remap = np.where(
        loc < 512, cc * 512 + loc,
        np.where(loc < 1024, 4096 + cc * 512 + (loc - 512),
                 8192 + cc * 256 + (loc - 1024))).astype(np.int64)

    # per (core, dst tile): unique sources split into opA (remap < A_ROWS,
    # gated on AG piece A), opB (piece B) and op2 (piece C);
    # oh[slot, dstoff] = edge count
    uniq = [[None] * NT for _ in range(C)]
    nA = np.zeros((C, NT), np.int64)
    nB = np.zeros((C, NT), np.int64)
    n2 = np.zeros((C, NT), np.int64)
    for c in range(C):
        for t in range(NT):
            g = c * NT + t
            sel = (dst >= g * 128) & (dst < (g + 1) * 128)
            s_e = src[sel]
            d_e = (dst[sel] - g * 128).astype(np.int64)
            us = np.unique(s_e)
            r = remap[us]
            usA, usB, us2 = us[r < A_ROWS], us[(r >= A_ROWS) & (r < AB_ROWS)], us[r >= AB_ROWS]
            uniq[c][t] = (usA, usB, us2, s_e, d_e)
            nA[c, t], nB[c, t], n2[c, t] = len(usA), len(usB), len(us2)

    cntAm = nA.max(axis=0)
    cntBm = nB.max(axis=0)
    cnt2m = n2.max(axis=0)
    TA = np.maximum(np.ceil(cntAm / 128).astype(np.int64), 1)
    TB = np.maximum(np.ceil(cntBm / 128).astype(np.int64), 1)
    T2 = np.maximum(np.ceil(cnt2m / 128).astype(np.int64), 1)
    T = TA + TB + T2
    bases = np.concatenate([[0], np.cumsum(T)])[:-1]
    ST = int(T.sum())

    gidx12 = np.full((C, 128, ST * 8), -1, np.int16)  # xg/x0r rows
    ohv = np.zeros((C, 128, ST, 128), np.float32)     # [slot, sub, dstoff]

    def fill(tbl, vals, col0):
        # value i lands at gather position col0*128 + i; idx table wraps
        # position j at [j % 16, j // 16], replicated over 8 row groups.
        if len(vals) == 0:
            return
        i = np.arange(len(vals))
        pos = col0 * 128 + i
        for r in range(8):
            tbl[16 * r + pos % 16, pos // 16] = vals

    for c in range(C):
        for t in range(NT):
            usA, usB, us2, s_e, d_e = uniq[c][t]
            b = int(bases[t])
            kA, kB = int(TA[t]), int(TB[t])
            # pad each op's index list to the cross-core max with idx 0
            # (the -1 tail after that is skipped via num_idxs_reg)
            slot_of = {}
            for us_i, cm, off in ((usA, cntAm[t], 0), (usB, cntBm[t], kA),
                                  (us2, cnt2m[t], kA + kB)):
                p = np.zeros(int(cm), np.int64)
                p[:len(us_i)] = us_i
                fill(gidx12[c], remap[p].astype(np.int16), b + off)
                for i, s in enumerate(us_i):
                    slot_of[int(s)] = (b + off + i // 128, i % 128)
            for s_i, d_i in zip(s_e, d_e):
                sub, p = slot_of[int(s_i)]
                ohv[c, p, sub, d_i] += 1.0

    x_pad = np.zeros((NP, D), np.float32)
    x_pad[:N] = np.asarray(x, dtype=np.float32)
    x0r = np.zeros((NP, D), np.float32)
    x0r[remap] = x_pad

    invdeg_sb = np.empty((C, 128, NT), np.float32)
    for c in range(C):
        invdeg_sb[c] = invdeg[c * NCORE:(c + 1) * NCORE].reshape(NT, 128).T

    xT0 = np.empty((C, 128, 4, NCORE), np.float32)
    for c in range(C):
        xT0[c] = x_pad[c * NCORE:(c + 1) * NCORE].reshape(NCORE, 4, 128).transpose(2, 1, 0)

    import ml_dtypes
    ohv = ohv.astype(ml_dtypes.bfloat16)
    return (x0r, gidx12, ohv, invdeg_sb, xT0,
            TA, TB, T2, bases, ST, cntAm, cntBm, cnt2m)


def _wsb(w):
    # [K, M] -> SBUF layout [128, K/128, M], bf16
    import ml_dtypes
    w = np.asarray(w, np.float32)
    return np.ascontiguousarray(
        w.reshape(w.shape[0] // 128, 128, w.shape[1]).transpose(1, 0, 2)
    ).astype(ml_dtypes.bfloat16)


def _bsb(b):
    # [M] -> [128, M/128]
    b = np.asarray(b, np.float32)
    return np.ascontiguousarray(b.reshape(b.shape[0] // 128, 128).T)


def _build_program(TA, TB, T2, bases, ST, cntAm, cntBm, cnt2m):
    TAMAX, TBMAX, T2MAX = int(TA.max()), int(TB.max()), int(T2.max())
    nc = bacc.Bacc(None, target_bir_lowering=False, debug=False, num_devices=C,
                   dynamic_dma_scratch_size=16384)

    x0_d = nc.declare_dram_parameter("x_full0", [NP, D], BF16, isOutput=False)
    gidx12_d = nc.declare_dram_parameter("gidx12", [128, ST * 8], I16, isOutput=False)
    oh_d = nc.declare_dram_parameter("ohv", [128, ST, 128], BF16, isOutput=False)
    invdeg_d = nc.declare_dram_parameter("invdeg", [128, NT], F32, isOutput=False)
    ident_d = nc.declare_dram_parameter("ident", [128, 128], F32, isOutput=False)
    identb_d = nc.declare_dram_parameter("identb", [128, 128], BF16, isOutput=False)
    zeros_d = nc.declare_dram_parameter("zeros", [128, 1], F32, isOutput=False)
    xT0_d = nc.declare_dram_parameter("xT0", [128, 4, NCORE], BF16, isOutput=False)
    w_d = {}
    for l in range(3):
        w_d[f"wl{l}"] = nc.declare_dram_parameter(f"wl{l}", [128, 4, H], BF16, isOutput=False)
        w_d[f"wr{l}"] = nc.declare_dram_parameter(f"wr{l}", [128, 4, H], BF16, isOutput=False)
        w_d[f"b{l}"] = nc.declare_dram_parameter(f"b{l}", [128, 4], F32, isOutput=False)
    wout_d = nc.declare_dram_parameter("wout", [128, 4, O], BF16, isOutput=False)
    bout_d = nc.declare_dram_parameter("bout", [128, 1], F32, isOutput=False)
    out_d = nc.declare_dram_parameter("out", [NCORE, O], F32, isOutput=True)

    xg = [None, nc.dram_tensor("xg1", [NP, D], BF16, addr_space="Shared"),
          nc.dram_tensor("xg2", [NP, D], BF16, addr_space="Shared")]
    xc = [None, nc.dram_tensor("xc1", [NCORE, D], BF16),
          nc.dram_tensor("xc2", [NCORE, D], BF16)]

    with tile.TileContext(nc) as tc:
        with tc.tile_pool(name="const", bufs=1) as constp, \
             tc.tile_pool(name="xT", bufs=2) as xTp, \
             tc.tile_pool(name="aggT", bufs=1) as aggTp, \
             tc.tile_pool(name="xsA", bufs=3) as xsAp, \
             tc.tile_pool(name="xsB", bufs=2) as xsBp, \
             tc.tile_pool(name="xs2", bufs=2) as xs2p, \
             tc.tile_pool(name="ohA", bufs=3) as ohAp, \
             tc.tile_pool(name="ohB", bufs=2) as ohBp, \
             tc.tile_pool(name="oh2", bufs=2) as oh2p, \
             tc.tile_pool(name="agg", bufs=2) as aggp, \
             tc.tile_pool(name="xnm", bufs=2) as xnmp, \
             tc.tile_pool(name="wts", bufs=2) as wp, \
             tc.tile_pool(name="dmy", bufs=3) as dmyp, \
             tc.tile_pool(name="pa", bufs=4, space="PSUM") as pap, \
             tc.tile_pool(name="pt", bufs=1, space="PSUM") as ptp, \
             tc.tile_pool(name="pd", bufs=2, space="PSUM") as pdp:

            # ---- load constants ----
            gidx12_sb = constp.tile([128, ST * 8], I16)
            nc.sync.dma_start(gidx12_sb[:], gidx12_d[:])
            invdeg_sb = constp.tile([128, NT], F32)
            nc.sync.dma_start(invdeg_sb[:], invdeg_d[:])
            ident = constp.tile([128, 128], F32)
            nc.sync.dma_start(ident[:], ident_d[:])
            identb = constp.tile([128, 128], BF16)
            nc.sync.dma_start(identb[:], identb_d[:])
            wsb = {}
            for l in range(3):
                wsb[f"b{l}"] = constp.tile([128, 4], F32, name=f"bsb{l}")
                nc.sync.dma_start(wsb[f"b{l}"][:], w_d[f"b{l}"][:])
            wout_sb = constp.tile([128, 4, O], BF16)
            nc.sync.dma_start(wout_sb[:], wout_d[:])
            bout_sb = constp.tile([128, 1], F32)
            nc.sync.dma_start(bout_sb[:], bout_d[:])

            xT_cur = xTp.tile([128, 4, NCORE], BF16)
            nc.sync.dma_start(xT_cur[:], xT0_d[:])

            gate_in = None
            for l in range(3):
                gidx = gidx12_sb
                aggT = aggTp.tile([128, 4, NCORE], BF16)
                xT_next = xTp.tile([128, 4, NCORE], BF16)
                wlr = wp.tile([128, 8, H], BF16, name="wlr")
                nc.sync.dma_start(wlr[:, 0:4, :], w_d[f"wl{l}"][:])
                nc.sync.dma_start(wlr[:, 4:8, :], w_d[f"wr{l}"][:])
                wl, wr, bb = wlr[:, 0:4, :], wlr[:, 4:8, :], wsb[f"b{l}"]

                pa_of = {}

                def do_gather_op(t, ne, nreal, b, xsp, ohp, xsmax, nm, src_ap,
                                 pa, start, stop):
                    xs = xsp.tile([128, xsmax, D], BF16, name=nm)
                    nc.vector.memset(xs[:, ne - 1, :], 0.0)
                    nc.gpsimd.dma_gather(
                        out_ap=xs[:, :ne, :], in_ap=src_ap,
                        idxs_ap=gidx[:, b * 8:(b + ne) * 8],
                        num_idxs=ne * 128, num_idxs_reg=nreal,
                        elem_size=D, single_packet=False)
                    oh = ohp.tile([128, xsmax, 128], BF16, name="oh" + nm)
                    nc.sync.dma_start(oh[:, :ne, :], oh_d[:, b:b + ne, :])
                    for e in range(ne):
                        nc.tensor.matmul(
                            pa[:], lhsT=oh[:, e, :], rhs=xs[:, e, :],
                            start=(start and e == 0), stop=(stop and e == ne - 1))

                def do_opA(t):
                    # gather srcs living in AG piece A; opens the tile's
                    # PSUM accumulation group (stays open through op2)
                    pa = pap.tile([128, D], F32, name="pa")
                    pa_of[t] = pa
                    src_ap = x0_d[0:A_ROWS, :] if l == 0 else xg[l][0:A_ROWS, :]
                    do_gather_op(t, int(TA[t]), int(cntAm[t]), int(bases[t]),
                                 xsAp, ohAp, TAMAX, "xsA", src_ap,
                                 pa, True, False)

                def do_opB(t):
                    # gather srcs living in AG piece B
                    src_ap = x0_d[0:AB_ROWS, :] if l == 0 else xg[l][0:AB_ROWS, :]
                    do_gather_op(t, int(TB[t]), int(cntBm[t]),
                                 int(bases[t]) + int(TA[t]),
                                 xsBp, ohBp, TBMAX, "xsB", src_ap,
                                 pa_of[t], False, False)

                def do_op2(t):
                    # gather srcs needing AG piece C, close the accumulation,
                    # scale by 1/deg, transpose to feat-major.
                    src_ap = x0_d[:] if l == 0 else xg[l][:]
                    pa = pa_of.pop(t)
                    do_gather_op(t, int(T2[t]), int(cnt2m[t]),
                                 int(bases[t]) + int(TA[t]) + int(TB[t]),
                                 xs2p, oh2p, T2MAX, "xs2", src_ap,
                                 pa, False, True)
                    agg = aggp.tile([128, D], F32, name="agg")
                    nc.scalar.activation(
                        agg[:], pa[:], mybir.ActivationFunctionType.Copy,
                        scale=invdeg_sb[:, t:t + 1])
                    for k in range(4):
                        pt = ptp.tile([128, 128], F32, name="pt")
                        nc.tensor.transpose(pt[:], agg[:, k * 128:(k + 1) * 128], ident[:])
                        nc.vector.tensor_copy(aggT[:, k, t * 128:(t + 1) * 128], pt[:])

                def do_dense_group(goff, gsz):
                    for m in range(4):
                        pd = pdp.tile([128, 512], F32, name="pd")
                        for k in range(4):
                            nc.tensor.matmul(
                                pd[:, :gsz],
                                lhsT=wl[:, k, m * 128:(m + 1) * 128],
                                rhs=aggT[:, k, goff:goff + gsz],
                                start=(k == 0), stop=False)
                        for k in range(4):
                            nc.tensor.matmul(
                                pd[:, :gsz],
                                lhsT=wr[:, k, m * 128:(m + 1) * 128],
                                rhs=xT_cur[:, k, goff:goff + gsz],
                                start=False, stop=(k == 3))
                        nc.scalar.activation(
                            xT_next[:, m, goff:goff + gsz], pd[:, :gsz],
                            mybir.ActivationFunctionType.Relu,
                            bias=bb[:, m:m + 1])
                    if l < 2:
                        for t in range(goff // 128, (goff + gsz) // 128):
                            xnm = xnmp.tile([128, D], BF16, name="xnm")
                            for k in range(4):
                                pt = ptp.tile([128, 128], BF16, name="ptx")
                                nc.tensor.transpose(
                                    pt[:], xT_next[:, k, t * 128:(t + 1) * 128],
                                    identb[:])
                                nc.vector.tensor_copy(xnm[:, k * 128:(k + 1) * 128], pt[:])
                            nc.sync.dma_start(xc[l + 1][t * 128:(t + 1) * 128, :], xnm[:])

                def do_final_group(t0, t1, xT_src):
                    goff, gsz = t0 * 128, (t1 - t0) * 128
                    pd = pdp.tile([128, 512], F32, name="pd")
                    for k in range(4):
                        nc.tensor.matmul(
                            pd[:, :gsz],
                            lhsT=wout_sb[:, k, :],
                            rhs=xT_src[:, k, goff:goff + gsz],
                            start=(k == 0), stop=(k == 3))
                    oT = aggp.tile([128, 512], F32)
                    nc.scalar.activation(
                        oT[:, :gsz], pd[:, :gsz],
                        mybir.ActivationFunctionType.Identity, bias=bout_sb[:, 0:1])
                    for tt in range(gsz // 128):
                        t = t0 + tt
                        pt = ptp.tile([128, 128], F32)
                        nc.tensor.transpose(pt[:], oT[:, tt * 128:(tt + 1) * 128], ident[:])
                        onm = xnmp.tile([128, O], F32)
                        nc.vector.tensor_copy(onm[:], pt[:])
                        nc.sync.dma_start(out_d[t * 128:(t + 1) * 128, :], onm[:])

                # Pool-queue order: each AG trigger is emitted only after the
                # NEXT group's op1 gathers, so a trigger blocked on its dense
                # group can never starve the gather stream (no head-of-line
                # cycle), while still firing ~2 groups earlier than end-of-layer.
                ag_bounds = [(0, 4096), (4096, 8192), (8192, 10240)]

                def do_ag(gi):
                    t0, t1 = GROUPS[gi]
                    with tc.high_priority():
                        do_dense_group(t0 * 128, (t1 - t0) * 128)
                        if l < 2:
                            lo, hi = ag_bounds[gi]
                            nc.gpsimd.collective_compute(
                                "AllGather", mybir.AluOpType.bypass,
                                replica_groups=[list(range(C))],
                                ins=[xc[l + 1][t0 * 128:t1 * 128, :]],
                                outs=[xg[l + 1][lo:hi, :]])
                            return None
                        do_final_group(t0, t1, xT_next)
                    return None

                # Gathers are split by the AG piece their sources live in, so
                # a layer's opA gathers can start as soon as piece A lands
                # (instead of waiting for A+B). AG triggers are emitted right
                # after the gather set their dense group needs plus a little
                # slack, so pieces start as early as their inputs allow.
                for t in range(0, 4):
                    do_opA(t)
                do_opA(4)
                for t in range(0, 4):
                    do_opB(t)
                for t in range(0, 4):
                    do_op2(t)
                do_ag(0)
                for t in range(5, 8):
                    do_opA(t)
                for t in range(4, 8):
                    do_opB(t)
                do_opA(8)
                for t in range(4, 8):
                    do_op2(t)
                do_ag(1)
                do_opA(9)
                do_opB(8)
                do_opB(9)
                for t in range(8, 10):
                    do_op2(t)
                do_ag(2)
                xT_cur = xT_next

    nc.compile()
    return nc


def _run(inputs, trace=False):
    x = inputs["x"]
    edge_index = inputs["edge_index"]
    (x0r, gidx12, ohv, invdeg_sb, xT0,
     TA, TB, T2, bases, ST, cntAm, cntBm, cnt2m) = _host_prep(x, edge_index)
    nc = _build_program(TA, TB, T2, bases, ST, cntAm, cntBm, cnt2m)

    import ml_dtypes
    shared = {
        "x_full0": x0r.astype(ml_dtypes.bfloat16),
        "wout": _wsb(inputs["w_out"]),
        "bout": np.asarray(inputs["b_out"], np.float32).reshape(128, 1),
        "ident": np.eye(128, dtype=np.float32),
        "identb": np.eye(128, dtype=np.float32).astype(ml_dtypes.bfloat16),
        "zeros": np.zeros((128, 1), np.float32),
    }
    for l in range(3):
        shared[f"wl{l}"] = _wsb(inputs[f"w_l{l}"])
        shared[f"wr{l}"] = _wsb(inputs[f"w_r{l}"])
        shared[f"b{l}"] = _bsb(inputs[f"b_l{l}"])

    in_maps = []
    for c in range(C):
        m = dict(shared)
        m["gidx12"] = np.ascontiguousarray(gidx12[c])
        m["ohv"] = np.ascontiguousarray(ohv[c])
        m["invdeg"] = np.ascontiguousarray(invdeg_sb[c])
        m["xT0"] = np.ascontiguousarray(xT0[c]).astype(ml_dtypes.bfloat16)
        in_maps.append(m)

    res = run_bass_kernel_spmd(nc, in_maps, list(range(C)), trace=trace)
    out = np.concatenate([res.results[c]["out"] for c in range(C)], axis=0)[:N]
    return out.astype(np.float32), res


def kernel(**inputs):
    out, _ = _run(inputs, trace=False)
    return out


def kernel_timed(**inputs):
    out, res = _run(inputs, trace=True)
    return out, res

